# revision 49
# baseline (speedup 1.0000x reference)
"""Trainium2 Bass kernel for nn_Attention_38276748542551.

Llama-style GQA attention block (DIM=4096, 32 q-heads, 8 kv-heads, hd=128,
b=2, s=2048, start_pos=0), tensor-parallel over heads across 8 NeuronCores:
each core owns 4 q-heads / 1 kv-head (wq/wk/wv output-dim shard, wo
input-dim shard) and computes a full [b*s, 4096] partial of the wo output;
the all-reduce is done on the host after gathering the 8 partials (partials
are written bf16 to halve the output DMA).

All matmuls run in bf16 (same 1 col/cycle PE rate as float32r but half the
LDWEIGHTS/SBUF/DMA traffic); accumulation is fp32 in PSUM. Measured rel-err
budget: bf16 operand quantization ~2e-3 vs the 2e-2 gate.

Device dataflow per core (everything feature-major, moving dim = 512 tokens):
  phase 1 (per 512-token block): Q/K/V projections with the weight k-tile
  stationary and x^T (host pre-transposed, bf16) moving -> outputs land
  [feat, tok] in PSUM -> rope applied by the DVE reading PSUM directly: the
  pair-swap is a partition half-rotation (even/odd deinterleave baked into
  the weight layout), done with partition-offset [64,512] multiplies -- no
  PE swap matmul, no ACT copy. Q^T goes to a resident SBUF tile (no DRAM
  roundtrip), K^T resident in SBUF, V PE-transposed to token-major bf16.
  phase 2 (per batch, per 512-token query block, per head): scores computed
  TRANSPOSED [sk, sq] per 128-sk-tile (lhsT = K^T tile, rhs = Q^T block,
  N=512) -> causal mask add on diagonal-band tiles -> exp (ACT, bf16 out) ->
  PV accumulation (lhsT = V tile, N=512) and row-sum accumulation (lhsT =
  ones column) on the PE -> denominator broadcast via a K=1 ones-matmul ->
  approx-reciprocal + multiply -> attn^T bf16, feature-major.
  phase 3 (per 128-token tile): wo projection (lhsT = attn^T, rhs = wo^T,
  8x N=512 chunks x 4 k-tiles) -> PSUM->SBUF copies alternate between the
  scalar and vector engines -> bf16 partial DMA'd out.
"""
import sys
import numpy as np
import ml_dtypes

sys.path.insert(0, "/opt/trn_rl_repo")

import concourse.bass as bass  # noqa: E402
import concourse.tile as tile  # noqa: E402
from concourse import bacc, mybir  # noqa: E402
from concourse import bass_utils  # noqa: E402

F32 = mybir.dt.float32
F32R = mybir.dt.float32r
BF16 = mybir.dt.bfloat16
AF = mybir.ActivationFunctionType
NPBF16 = ml_dtypes.bfloat16

DIM = 4096
NK = DIM // 128          # contraction k-tiles (32)
NKQ = 4                  # k quarters
KPQ = NK // NKQ          # k-tiles per quarter (8)
HD = 128                 # head dim
NH_LOC = 4               # q heads per core
QDIM = NH_LOC * HD       # 512
KVDIM = 2 * HD           # K and V projected together, 256
N_CORES = 8
SOFTMAX_SCALE = 1.0 / np.sqrt(HD)


def build_nc(B=2, S=2048):
    """Build the per-core Bass program (identical across cores; data differs)."""
    NT = B * S // 128            # 128-token tiles total
    TPB = S // 128               # 128-token tiles per batch
    NQB = S // 512               # 512-token blocks per batch
    NTG = B * NQB                # 512-token blocks total

    nc = bacc.Bacc("TRN2", target_bir_lowering=False, debug=False,
                   enable_asserts=False, num_devices=N_CORES)

    # x_t laid out partition-contiguous on host: [g, kq, p, k, t]
    x_t = nc.dram_tensor("x_t", [NTG, NKQ, 128, KPQ, 512], BF16, kind="ExternalInput").ap()
    # weights laid out [p, k, n] on host so DMA lines are contiguous per partition
    wq_t = nc.dram_tensor("wq_t", [128, NK, QDIM], BF16, kind="ExternalInput").ap()
    wkv_t = nc.dram_tensor("wkv_t", [128, NK, KVDIM], BF16, kind="ExternalInput").ap()
    wo_t = nc.dram_tensor("wo_t", [QDIM, DIM], BF16, kind="ExternalInput").ap()
    cct_d = nc.dram_tensor("cct", [128, S], BF16, kind="ExternalInput").ap()
    sst_d = nc.dram_tensor("sst", [128, S], BF16, kind="ExternalInput").ap()
    ident_d = nc.dram_tensor("ident", [128, 128], BF16, kind="ExternalInput").ap()
    ones_d = nc.dram_tensor("ones", [128, 128], F32, kind="ExternalInput").ap()
    onesb_d = nc.dram_tensor("onesb", [128, 128], BF16, kind="ExternalInput").ap()
    masks_d = nc.dram_tensor("masks", [128, 4, 512], F32, kind="ExternalInput").ap()
    out_d = nc.dram_tensor("out", [B * S, DIM], BF16, kind="ExternalOutput").ap()

    with tile.TileContext(nc) as tc:
        with tc.tile_pool(name="singles", bufs=1) as singles:
            ident = singles.tile([128, 128], BF16)
            ones_r = singles.tile([128, 128], F32R)
            ones_b = singles.tile([128, 128], BF16)
            kt_sb = singles.tile([128, NT, 128], BF16)    # K^T: [hd, tile, tok]
            v_sb = singles.tile([128, NT, 128], BF16)     # V: [tok, tile, hd]
            qt_sb = singles.tile([128, NTG, NH_LOC, 512], BF16)  # Q^T resident
            cct_sb = singles.tile([128, S], BF16)
            sst_sb = singles.tile([128, S], BF16)

            # ---------------- phase 1: projections + rope (feature-major) ----------------
            with tc.tile_pool(name="p1w", bufs=1) as p1w, \
                 tc.tile_pool(name="p1", bufs=4) as p1, \
                 tc.tile_pool(name="p1r", bufs=3) as p1r, \
                 tc.tile_pool(name="ps_acc", bufs=6, space="PSUM") as ps_accp, \
                 tc.tile_pool(name="ps_misc", bufs=2, space="PSUM") as ps_miscp:

                def load_xs(g, kq):
                    t_ = p1.tile([128, KPQ, 512], BF16, tag="xs")
                    nc.sync.dma_start(out=t_, in_=x_t[g, kq])
                    return t_

                wq_sb = p1w.tile([128, NK, QDIM], BF16)
                wkv_sb = p1w.tile([128, NK, KVDIM], BF16)

                def load_wq(k0, k1):
                    nc.sync.dma_start(
                        out=wq_sb[:, k0:k1, :], in_=wq_t[:, k0:k1, :])

                def load_wkv(k0, k1):
                    nc.sync.dma_start(
                        out=wkv_sb[:, k0:k1, :], in_=wkv_t[:, k0:k1, :])

                # interleave the first xs quarter with the weight chunk loads so
                # the first matmul can start as early as possible
                xs00 = p1.tile([128, KPQ, 512], BF16, tag="xs")
                nc.sync.dma_start(out=xs00[:, 0:1, :], in_=x_t[0, 0, :, 0:1])
                load_wq(0, 1)
                load_wkv(0, 1)
                nc.sync.dma_start(out=xs00[:, 1:3, :], in_=x_t[0, 0, :, 1:3])
                load_wq(1, 3)
                load_wkv(1, 3)
                nc.sync.dma_start(out=xs00[:, 3:8, :], in_=x_t[0, 0, :, 3:8])
                load_wq(3, 8)
                load_wkv(3, 8)
                # deadline-ordered: each xs quarter + the weight k-tiles it
                # needs, so concurrent DMA queues don't starve the urgent ones
                xs_pre = [xs00]
                xs_pre.append(load_xs(0, 1))
                load_wq(8, 16)
                load_wkv(8, 16)
                xs_pre.append(load_xs(0, 2))
                load_wq(16, 24)
                load_wkv(16, 24)
                load_wq(24, 32)
                load_wkv(24, 32)
                nc.sync.dma_start(out=cct_sb, in_=cct_d)
                nc.sync.dma_start(out=sst_sb, in_=sst_d)
                nc.sync.dma_start(out=ident, in_=ident_d)
                nc.sync.dma_start(out=ones_r, in_=ones_d.bitcast(F32R))
                nc.sync.dma_start(out=ones_b, in_=onesb_d)

                nload = 3   # next (g*NKQ+kq) index to load; keep 2-3 in flight
                for g in range(NTG):
                    pos = (g % NQB) * 512
                    acc = [ps_accp.tile([128, 512], F32, tag="acc", name=f"acc{g}_{j}") for j in range(6)]
                    for kq in range(NKQ):
                        xs = xs_pre.pop(0)
                        if nload < NTG * NKQ:
                            xs_pre.append(load_xs(nload // NKQ, nload % NKQ))
                            nload += 1
                        for k in range(KPQ):
                            kt = kq * KPQ + k
                            st = (kt == 0)
                            sp = (kt == NK - 1)
                            for h in range(NH_LOC):
                                nc.tensor.matmul(acc[h], wq_sb[:, kt, h * 128:(h + 1) * 128],
                                                 xs[:, k, :], start=st, stop=sp)
                            nc.tensor.matmul(acc[4], wkv_sb[:, kt, 0:128],
                                             xs[:, k, :], start=st, stop=sp)
                            nc.tensor.matmul(acc[5], wkv_sb[:, kt, 128:256],
                                             xs[:, k, :], start=st, stop=sp)

                    # Free the PSUM accumulators FAST with split ACT/DVE bf16
                    # staging copies (so the next block's matmuls aren't blocked
                    # on the rope chain), then rope from the bf16 staging.
                    stage = [p1r.tile([128, 512], BF16, tag=f"st{j}", name=f"st{g}_{j}")
                             for j in range(6)]
                    for j in range(6):
                        if j % 2 == 0:
                            nc.scalar.copy(stage[j], acc[j])
                        else:
                            nc.vector.tensor_copy(stage[j], acc[j])
                    # rope Q (4 heads) + K on the DVE in bf16 (2x mode).
                    # feature layout per head: [0:64]=even pairs, [64:128]=odd.
                    # out_e = e*c - o*s ; out_o = o*c + e*s. sst is stored
                    # half-swapped ([+s; -s]) so each multiply reads both SBUF
                    # inputs at the same base partition (HW constraint) and the
                    # half-rotation happens via the shifted output write.
                    cct = cct_sb[:, pos:pos + 512]
                    sst = sst_sb[:, pos:pos + 512]
                    for j in range(5):   # 0..3 = q heads, 4 = K
                        t1 = p1r.tile([128, 512], BF16, tag="t1")
                        nc.vector.tensor_mul(t1, stage[j], cct)
                        t2 = p1r.tile([128, 512], BF16, tag="t2")
                        nc.vector.tensor_mul(t2[0:64, :], stage[j][64:128, :], sst[64:128, :])
                        nc.vector.tensor_mul(t2[64:128, :], stage[j][0:64, :], sst[0:64, :])
                        if j < NH_LOC:
                            nc.vector.tensor_add(qt_sb[:, g, j, :], t1, t2)
                        else:
                            nc.vector.tensor_add(
                                kt_sb[:, 4 * g:4 * g + 4, :].rearrange("p a t -> p (a t)"),
                                t1, t2)
                    for r in range(4):
                        ps_vt = ps_miscp.tile([128, 512], BF16, tag="misc")
                        nc.tensor.transpose(ps_vt[:, 0:128], stage[5][:, r * 128:(r + 1) * 128], ident)
                        nc.scalar.copy(v_sb[:, 4 * g + r, :], ps_vt[:, 0:128])

            # ------------- phase 2/3: attention (transposed scores) + wo -------------
            with tc.tile_pool(name="p2w", bufs=1) as p2w, \
                 tc.tile_pool(name="p2", bufs=2) as p2, \
                 tc.tile_pool(name="p2e", bufs=6) as p2e, \
                 tc.tile_pool(name="p2l", bufs=4) as p2l, \
                 tc.tile_pool(name="ps_s", bufs=4, space="PSUM") as ps_sp, \
                 tc.tile_pool(name="ps_o", bufs=1, space="PSUM") as ps_op, \
                 tc.tile_pool(name="ps_l", bufs=1, space="PSUM") as ps_lp, \
                 tc.tile_pool(name="ps_w", bufs=2, space="PSUM") as ps_wp:
                masks_sb = p2w.tile([128, 4, 512], F32)
                nc.sync.dma_start(out=masks_sb, in_=masks_d)
                # one PSUM bank, two rowsum rows: heads alternate rows so a
                # head's first rowsum never waits on the previous head's lr copy
                ps_l2 = ps_lp.tile([64, 512], F32, tag="ps_l")
                wo_sb = p2w.tile([128, NH_LOC, DIM], BF16)
                for kk in range(NH_LOC):   # chunked so the first wo can start early
                    nc.sync.dma_start(
                        out=wo_sb[:, kk, :],
                        in_=wo_t[kk * 128:(kk + 1) * 128, :])

                # Attention and wo are emitted as generators and interleaved:
                # each block's attention steps alternate with the PREVIOUS
                # block's wo chunks so the scheduler's priorities (emission
                # order) match the intended concurrency.
                def attn_block(b, qb, holder):
                    g = b * NQB + qb
                    nt = 4 * (qb + 1)
                    attn_t = p2.tile([128, NH_LOC, 4, 128], BF16, tag="attn_t",
                                     name=f"attn_t{g}")
                    holder["attn_t"] = attn_t
                    holder["b"] = b
                    holder["qb"] = qb
                    for h in range(NH_LOC):
                        ps_o = ps_op.tile([128, 512], F32, tag="ps_o",
                                          name=f"ps_o{g}_{h}")
                        lrow = 32 * (h % 2)
                        ps_l = ps_l2[lrow:lrow + 1, :]
                        for t in range(nt):
                            # diagonal tile v: columns [0, 128v) fully masked ->
                            # skipped everywhere; triangular corner gets the mask
                            v = t - 4 * qb
                            q0 = 128 * v if v > 0 else 0
                            ps_s = ps_sp.tile([128, 512], F32, tag="ps_s",
                                              name=f"ps_s{g}_{h}_{t}")
                            nc.tensor.matmul(ps_s[:, q0:512],
                                             kt_sb[:, b * TPB + t, :],
                                             qt_sb[:, g, h, q0:512],
                                             start=True, stop=True)
                            if v >= 0:
                                nc.vector.tensor_add(
                                    ps_s[:, q0:q0 + 128], ps_s[:, q0:q0 + 128],
                                    masks_sb[:, v, q0:q0 + 128])
                            et = p2e.tile([128, 512], BF16, tag="et",
                                          name=f"et{g}_{h}_{t}")
                            nc.scalar.activation(et[:, q0:512], ps_s[:, q0:512],
                                                 AF.Exp, scale=SOFTMAX_SCALE)
                            nc.tensor.matmul(ps_o[:, q0:512],
                                             v_sb[:, b * TPB + t, :],
                                             et[:, q0:512],
                                             start=(t == 0), stop=(t == nt - 1),
                                             skip_group_check=True)
                            nc.tensor.matmul(ps_l[0:1, q0:512], ones_b[:, 0:1],
                                             et[:, q0:512],
                                             start=(t == 0), stop=(t == nt - 1),
                                             skip_group_check=True)
                            yield
                        # drain ps_o fast via ACT so its single bank frees for
                        # the next head; normalize from SBUF afterwards
                        ao = p2l.tile([128, 512], BF16, tag="ao", name=f"ao{g}_{h}")
                        nc.scalar.copy(ao, ps_o)
                        lr = p2l.tile([1, 512], F32R, tag="lr", name=f"lr{g}_{h}")
                        nc.scalar.copy(lr, ps_l)
                        ps_b = ps_wp.tile([128, 512], F32, tag="ps_w",
                                          name=f"ps_b{g}_{h}")
                        nc.tensor.matmul(ps_b, ones_r[0:1, :], lr, start=True, stop=True)
                        rb = p2l.tile([128, 512], F32, tag="rb", name=f"rb{g}_{h}")
                        nc.vector.reciprocal_approx_fast(out=rb, in_=ps_b)
                        nc.vector.tensor_mul(
                            attn_t[:, h].rearrange("p r t -> p (r t)"), ao, rb)
                        yield

                def wo_block(holder):
                    b, qb, attn_t = holder["b"], holder["qb"], holder["attn_t"]
                    for r in range(4):
                        tt = b * TPB + qb * 4 + r
                        o_sb = p2.tile([128, DIM], BF16, tag="o_sb",
                                       name=f"o_sb{b}_{qb}_{r}")
                        for n in range(DIM // 512):
                            ps_w = ps_wp.tile([128, 512], F32, tag="ps_w",
                                              name=f"ps_w{b}_{qb}_{r}_{n}")
                            for kk in range(NH_LOC):
                                nc.tensor.matmul(ps_w, attn_t[:, kk, r, :],
                                                 wo_sb[:, kk, n * 512:(n + 1) * 512],
                                                 start=(kk == 0), stop=(kk == NH_LOC - 1))
                            # alternate PSUM->SBUF drain between DVE and ACT
                            if n % 2 == 0:
                                nc.vector.tensor_copy(o_sb[:, n * 512:(n + 1) * 512], ps_w)
                            else:
                                nc.scalar.copy(o_sb[:, n * 512:(n + 1) * 512], ps_w)
                            if n == 3:   # first half out early: shorter drain tail
                                nc.sync.dma_start(
                                    out=out_d[tt * 128:(tt + 1) * 128, 0:2048],
                                    in_=o_sb[:, 0:2048])
                            yield
                        nc.sync.dma_start(out=out_d[tt * 128:(tt + 1) * 128, 2048:4096],
                                          in_=o_sb[:, 2048:4096])
                        yield

                # qb descending: phase 2 opens with 36 mask-free matmuls
                # (qb=3 head 0) so the masks DMA latency is hidden
                prev_wo = None
                for b in range(B):
                    for qb in reversed(range(NQB)):
                        holder = {}
                        ag = attn_block(b, qb, holder)
                        for _ in ag:
                            if prev_wo is not None:
                                next(prev_wo, None)
                        if prev_wo is not None:
                            for _ in prev_wo:
                                pass
                        prev_wo = wo_block(holder)
                for _ in prev_wo:
                    pass

    nc.compile()
    return nc


def host_prepare(x, wq, wk, wv, wo, freqs_cos, freqs_sin, B, S):
    """Build per-core in_maps. Weights nn.Linear-style [out, in]."""
    NQB = S // 512
    NTG = B * NQB
    n_heads = wq.shape[0] // HD
    n_kv = wk.shape[0] // HD
    hpc = n_heads // N_CORES       # q heads per core (4)
    kpc = n_kv // N_CORES          # kv heads per core (1)

    # deinterleave rope pairs: feature order (2i) first then (2i+1), per head
    de = np.concatenate([np.arange(0, HD, 2), np.arange(1, HD, 2)])

    xf = np.ascontiguousarray(x.reshape(B * S, DIM))
    # x^T tiled, partition-contiguous: [g, kq, p, k, t]
    x_t = np.ascontiguousarray(
        xf.T.reshape(NKQ, KPQ, 128, NTG, 512).transpose(3, 0, 2, 1, 4)).astype(NPBF16)

    cos = np.repeat(freqs_cos, 2, axis=1)   # [S, 128] interleaved dup
    sin = np.repeat(freqs_sin, 2, axis=1)
    cc = cos[:, de]                                             # deinterleaved
    ss = sin.copy()
    ss[:, 1::2] *= -1.0                     # half-swapped: [+sin; -sin]
    ss = ss[:, de]
    cct = np.ascontiguousarray(cc.T).astype(NPBF16)             # [128, S]
    sst = np.ascontiguousarray(ss.T).astype(NPBF16)

    ident = np.eye(128, dtype=NPBF16)
    ones = np.ones((128, 128), dtype=np.float32)
    onesb = np.ones((128, 128), dtype=NPBF16)
    # transposed-orientation causal masks: scores^T [sk within tile, sq in 512]
    r_idx = np.arange(128)[:, None]
    j_idx = np.arange(512)[None, :]
    masks = np.ascontiguousarray(np.stack([
        np.where(v * 128 + r_idx <= j_idx, 0.0, -1e30).astype(np.float32)
        for v in range(4)]).transpose(1, 0, 2))   # [128, 4, 512]

    in_maps = []
    for cidx in range(N_CORES):
        qs = slice(cidx * hpc * HD, (cidx + 1) * hpc * HD)
        ks = slice(cidx * kpc * HD, (cidx + 1) * kpc * HD)
        wq_c = wq[qs].reshape(hpc, HD, DIM)[:, de, :].reshape(hpc * HD, DIM)
        wk_c = wk[ks].reshape(kpc, HD, DIM)[:, de, :].reshape(kpc * HD, DIM)
        wv_c = wv[ks]
        wkv_c = np.concatenate([wk_c, wv_c], axis=0)
        wo_c = wo[:, qs]
        # weights [out, in] -> [p, ktile, out] partition-contiguous
        wq_pk = np.ascontiguousarray(
            wq_c.T.reshape(NK, 128, hpc * HD).transpose(1, 0, 2)).astype(NPBF16)
        wkv_pk = np.ascontiguousarray(
            wkv_c.T.reshape(NK, 128, KVDIM).transpose(1, 0, 2)).astype(NPBF16)
        in_maps.append({
            "x_t": x_t,
            "wq_t": wq_pk,
            "wkv_t": wkv_pk,
            "wo_t": np.ascontiguousarray(wo_c.T).astype(NPBF16),
            "cct": cct,
            "sst": sst,
            "ident": ident,
            "ones": ones,
            "onesb": onesb,
            "masks": masks,
        })
    return in_maps


_CACHE = {}


def run(inputs, trace=False, trace_cores=None):
    x = np.asarray(inputs["x"], dtype=np.float32)
    B, S, _ = x.shape
    key = (B, S)
    if key not in _CACHE:
        _CACHE[key] = build_nc(B, S)
    nc = _CACHE[key]
    in_maps = host_prepare(
        x, np.asarray(inputs["wq"], np.float32), np.asarray(inputs["wk"], np.float32),
        np.asarray(inputs["wv"], np.float32), np.asarray(inputs["wo"], np.float32),
        np.asarray(inputs["freqs_cos"], np.float32),
        np.asarray(inputs["freqs_sin"], np.float32), B, S)
    res = bass_utils.run_bass_kernel_spmd(
        nc, in_maps, core_ids=list(range(N_CORES)), trace=trace,
        trace_cores=trace_cores)
    acc = np.zeros((B * S, DIM), dtype=np.float64)
    for r in res.results:
        acc += r["out"].astype(np.float64)
    out = acc.astype(np.float32).reshape(B, S, DIM)
    return out, res


def kernel(**inputs) -> np.ndarray:
    assert int(inputs.get("start_pos", 0)) == 0
    out, _ = run(inputs, trace=False)
    return out


# revision 51
# speedup vs baseline: 1.0203x; 1.0203x over previous
"""Trainium2 Bass kernel for nn_Attention_38276748542551.

Llama-style GQA attention block (DIM=4096, 32 q-heads, 8 kv-heads, hd=128,
b=2, s=2048, start_pos=0), tensor-parallel over heads across 8 NeuronCores:
each core owns 4 q-heads / 1 kv-head (wq/wk/wv output-dim shard, wo
input-dim shard) and computes a full [b*s, 4096] partial of the wo output;
the all-reduce is done on the host after gathering the 8 partials (partials
are written bf16 to halve the output DMA).

All matmuls run in bf16 (same 1 col/cycle PE rate as float32r but half the
LDWEIGHTS/SBUF/DMA traffic); accumulation is fp32 in PSUM. Measured rel-err
budget: bf16 operand quantization ~2e-3 vs the 2e-2 gate.

Device dataflow per core (everything feature-major, moving dim = 512 tokens):
  phase 1 (per 512-token block): Q/K/V projections with the weight k-tile
  stationary and x^T (host pre-transposed, bf16) moving -> outputs land
  [feat, tok] in PSUM -> rope applied by the DVE reading PSUM directly: the
  pair-swap is a partition half-rotation (even/odd deinterleave baked into
  the weight layout), done with partition-offset [64,512] multiplies -- no
  PE swap matmul, no ACT copy. Q^T goes to a resident SBUF tile (no DRAM
  roundtrip), K^T resident in SBUF, V PE-transposed to token-major bf16.
  phase 2 (per batch, per 512-token query block, per head): scores computed
  TRANSPOSED [sk, sq] per 128-sk-tile (lhsT = K^T tile, rhs = Q^T block,
  N=512) -> causal mask add on diagonal-band tiles -> exp (ACT, bf16 out) ->
  PV accumulation (lhsT = V tile, N=512) and row-sum accumulation (lhsT =
  ones column) on the PE -> denominator broadcast via a K=1 ones-matmul ->
  approx-reciprocal + multiply -> attn^T bf16, feature-major.
  phase 3 (per 128-token tile): wo projection (lhsT = attn^T, rhs = wo^T,
  8x N=512 chunks x 4 k-tiles) -> PSUM->SBUF copies alternate between the
  scalar and vector engines -> bf16 partial DMA'd out.
"""
import sys
import numpy as np
import ml_dtypes

sys.path.insert(0, "/opt/trn_rl_repo")

import concourse.bass as bass  # noqa: E402
import concourse.tile as tile  # noqa: E402
from concourse import bacc, mybir  # noqa: E402
from concourse import bass_utils  # noqa: E402

F32 = mybir.dt.float32
F32R = mybir.dt.float32r
BF16 = mybir.dt.bfloat16
AF = mybir.ActivationFunctionType
NPBF16 = ml_dtypes.bfloat16

DIM = 4096
NK = DIM // 128          # contraction k-tiles (32)
NKQ = 4                  # k quarters
KPQ = NK // NKQ          # k-tiles per quarter (8)
HD = 128                 # head dim
NH_LOC = 4               # q heads per core
QDIM = NH_LOC * HD       # 512
KVDIM = 2 * HD           # K and V projected together, 256
N_CORES = 8
SOFTMAX_SCALE = 1.0 / np.sqrt(HD)


def build_nc(B=2, S=2048):
    """Build the per-core Bass program (identical across cores; data differs)."""
    NT = B * S // 128            # 128-token tiles total
    TPB = S // 128               # 128-token tiles per batch
    NQB = S // 512               # 512-token blocks per batch
    NTG = B * NQB                # 512-token blocks total

    nc = bacc.Bacc("TRN2", target_bir_lowering=False, debug=False,
                   enable_asserts=False, num_devices=N_CORES)

    # x_t laid out partition-contiguous on host: [g, kq, p, k, t]
    x_t = nc.dram_tensor("x_t", [NTG, NKQ, 128, KPQ, 512], BF16, kind="ExternalInput").ap()
    # weights laid out [p, k, n] on host so DMA lines are contiguous per partition
    wq_t = nc.dram_tensor("wq_t", [128, NK, QDIM], BF16, kind="ExternalInput").ap()
    wkv_t = nc.dram_tensor("wkv_t", [128, NK, KVDIM], BF16, kind="ExternalInput").ap()
    wo_t = nc.dram_tensor("wo_t", [QDIM, DIM], BF16, kind="ExternalInput").ap()
    cct_d = nc.dram_tensor("cct", [128, S], BF16, kind="ExternalInput").ap()
    sst_d = nc.dram_tensor("sst", [128, S], BF16, kind="ExternalInput").ap()
    ident_d = nc.dram_tensor("ident", [128, 128], BF16, kind="ExternalInput").ap()
    ones_d = nc.dram_tensor("ones", [128, 128], F32, kind="ExternalInput").ap()
    onesb_d = nc.dram_tensor("onesb", [128, 128], BF16, kind="ExternalInput").ap()
    masks_d = nc.dram_tensor("masks", [128, 4, 512], F32, kind="ExternalInput").ap()
    out_d = nc.dram_tensor("out", [B * S, DIM], BF16, kind="ExternalOutput").ap()

    with tile.TileContext(nc) as tc:
        with tc.tile_pool(name="singles", bufs=1) as singles:
            ident = singles.tile([128, 128], BF16)
            ones_r = singles.tile([128, 128], F32R)
            ones_b = singles.tile([128, 128], BF16)
            kt_sb = singles.tile([128, NT, 128], BF16)    # K^T: [hd, tile, tok]
            v_sb = singles.tile([128, NT, 128], BF16)     # V: [tok, tile, hd]
            qt_sb = singles.tile([128, NTG, NH_LOC, 512], BF16)  # Q^T resident
            cct_sb = singles.tile([128, S], BF16)
            sst_sb = singles.tile([128, S], BF16)

            # ---------------- phase 1: projections + rope (feature-major) ----------------
            with tc.tile_pool(name="p1w", bufs=1) as p1w, \
                 tc.tile_pool(name="p1", bufs=4) as p1, \
                 tc.tile_pool(name="p1r", bufs=3) as p1r, \
                 tc.tile_pool(name="ps_acc", bufs=6, space="PSUM") as ps_accp, \
                 tc.tile_pool(name="ps_misc", bufs=2, space="PSUM") as ps_miscp:

                def load_xs(g, kq):
                    t_ = p1.tile([128, KPQ, 512], BF16, tag="xs")
                    nc.sync.dma_start(out=t_, in_=x_t[g, kq])
                    return t_

                wq_sb = p1w.tile([128, NK, QDIM], BF16)
                wkv_sb = p1w.tile([128, NK, KVDIM], BF16)

                def load_wq(k0, k1):
                    nc.sync.dma_start(
                        out=wq_sb[:, k0:k1, :], in_=wq_t[:, k0:k1, :])

                def load_wkv(k0, k1):
                    nc.sync.dma_start(
                        out=wkv_sb[:, k0:k1, :], in_=wkv_t[:, k0:k1, :])

                # interleave the first xs quarter with the weight chunk loads so
                # the first matmul can start as early as possible
                xs00 = p1.tile([128, KPQ, 512], BF16, tag="xs")
                nc.sync.dma_start(out=xs00[:, 0:1, :], in_=x_t[0, 0, :, 0:1])
                load_wq(0, 1)
                load_wkv(0, 1)
                nc.sync.dma_start(out=xs00[:, 1:3, :], in_=x_t[0, 0, :, 1:3])
                load_wq(1, 3)
                load_wkv(1, 3)
                nc.sync.dma_start(out=xs00[:, 3:8, :], in_=x_t[0, 0, :, 3:8])
                load_wq(3, 8)
                load_wkv(3, 8)
                # deadline-ordered: each xs quarter + the weight k-tiles it
                # needs, so concurrent DMA queues don't starve the urgent ones
                xs_pre = [xs00]
                xs_pre.append(load_xs(0, 1))
                load_wq(8, 16)
                load_wkv(8, 16)
                xs_pre.append(load_xs(0, 2))
                load_wq(16, 24)
                load_wkv(16, 24)
                load_wq(24, 32)
                load_wkv(24, 32)
                nc.sync.dma_start(out=cct_sb, in_=cct_d)
                nc.sync.dma_start(out=sst_sb, in_=sst_d)
                nc.sync.dma_start(out=ident, in_=ident_d)
                nc.sync.dma_start(out=ones_r, in_=ones_d.bitcast(F32R))
                nc.sync.dma_start(out=ones_b, in_=onesb_d)

                nload = 3   # next (g*NKQ+kq) index to load; keep 2-3 in flight
                for g in range(NTG):
                    pos = (g % NQB) * 512
                    acc = [ps_accp.tile([128, 512], F32, tag="acc", name=f"acc{g}_{j}") for j in range(6)]
                    for kq in range(NKQ):
                        xs = xs_pre.pop(0)
                        if nload < NTG * NKQ:
                            xs_pre.append(load_xs(nload // NKQ, nload % NKQ))
                            nload += 1
                        for k in range(KPQ):
                            kt = kq * KPQ + k
                            st = (kt == 0)
                            sp = (kt == NK - 1)
                            for h in range(NH_LOC):
                                nc.tensor.matmul(acc[h], wq_sb[:, kt, h * 128:(h + 1) * 128],
                                                 xs[:, k, :], start=st, stop=sp)
                            nc.tensor.matmul(acc[4], wkv_sb[:, kt, 0:128],
                                             xs[:, k, :], start=st, stop=sp)
                            nc.tensor.matmul(acc[5], wkv_sb[:, kt, 128:256],
                                             xs[:, k, :], start=st, stop=sp)

                    # Free the PSUM accumulators FAST with split ACT/DVE bf16
                    # staging copies (so the next block's matmuls aren't blocked
                    # on the rope chain), then rope from the bf16 staging.
                    stage = [p1r.tile([128, 512], BF16, tag=f"st{j}", name=f"st{g}_{j}")
                             for j in range(6)]
                    for j in range(6):
                        if j % 2 == 0:
                            nc.scalar.copy(stage[j], acc[j])
                        else:
                            nc.vector.tensor_copy(stage[j], acc[j])
                    # rope Q (4 heads) + K on the DVE in bf16 (2x mode).
                    # feature layout per head: [0:64]=even pairs, [64:128]=odd.
                    # out_e = e*c - o*s ; out_o = o*c + e*s. sst is stored
                    # half-swapped ([+s; -s]) so each multiply reads both SBUF
                    # inputs at the same base partition (HW constraint) and the
                    # half-rotation happens via the shifted output write.
                    cct = cct_sb[:, pos:pos + 512]
                    sst = sst_sb[:, pos:pos + 512]
                    for j in range(5):   # 0..3 = q heads, 4 = K
                        t1 = p1r.tile([128, 512], BF16, tag="t1")
                        nc.vector.tensor_mul(t1, stage[j], cct)
                        t2 = p1r.tile([128, 512], BF16, tag="t2")
                        nc.vector.tensor_mul(t2[0:64, :], stage[j][64:128, :], sst[64:128, :])
                        nc.vector.tensor_mul(t2[64:128, :], stage[j][0:64, :], sst[0:64, :])
                        if j < NH_LOC:
                            nc.vector.tensor_add(qt_sb[:, g, j, :], t1, t2)
                        else:
                            nc.vector.tensor_add(
                                kt_sb[:, 4 * g:4 * g + 4, :].rearrange("p a t -> p (a t)"),
                                t1, t2)
                    for r in range(4):
                        ps_vt = ps_miscp.tile([128, 512], BF16, tag="misc")
                        nc.tensor.transpose(ps_vt[:, 0:128], stage[5][:, r * 128:(r + 1) * 128], ident)
                        nc.scalar.copy(v_sb[:, 4 * g + r, :], ps_vt[:, 0:128])

            # ------------- phase 2/3: attention (transposed scores) + wo -------------
            with tc.tile_pool(name="p2w", bufs=1) as p2w, \
                 tc.tile_pool(name="p2", bufs=2) as p2, \
                 tc.tile_pool(name="p2e", bufs=4) as p2e, \
                 tc.tile_pool(name="p2l", bufs=4) as p2l, \
                 tc.tile_pool(name="ps_s", bufs=2, space="PSUM") as ps_sp, \
                 tc.tile_pool(name="ps_o", bufs=1, space="PSUM") as ps_op, \
                 tc.tile_pool(name="ps_l", bufs=1, space="PSUM") as ps_lp, \
                 tc.tile_pool(name="ps_w", bufs=2, space="PSUM") as ps_wp:
                masks_sb = p2w.tile([128, 4, 512], F32)
                nc.sync.dma_start(out=masks_sb, in_=masks_d)
                # one PSUM bank, two rowsum rows: heads alternate rows so a
                # head's first rowsum never waits on the previous head's lr copy
                ps_l2 = ps_lp.tile([64, 512], F32, tag="ps_l")
                wo_sb = p2w.tile([128, NH_LOC, DIM], BF16)
                for kk in range(NH_LOC):   # chunked so the first wo can start early
                    nc.sync.dma_start(
                        out=wo_sb[:, kk, :],
                        in_=wo_t[kk * 128:(kk + 1) * 128, :])

                # Attention and wo are emitted as generators and interleaved:
                # each block's attention steps alternate with the PREVIOUS
                # block's wo chunks so the scheduler's priorities (emission
                # order) match the intended concurrency.
                def attn_block(b, qb, holder):
                    g = b * NQB + qb
                    nt = 4 * (qb + 1)
                    attn_t = p2.tile([128, NH_LOC, 4, 128], BF16, tag="attn_t",
                                     name=f"attn_t{g}")
                    holder["attn_t"] = attn_t
                    holder["b"] = b
                    holder["qb"] = qb
                    def q0_of(t):
                        v = t - 4 * qb
                        return 128 * v if v > 0 else 0

                    for h in range(NH_LOC):
                        ps_o = ps_op.tile([128, 512], F32, tag="ps_o",
                                          name=f"ps_o{g}_{h}")
                        lrow = 32 * (h % 2)
                        ps_l = ps_l2[lrow:lrow + 1, :]

                        # k-tiles in PAIRS: one exp per pair over a 2-bank
                        # [128,1024] PSUM read halves the ACT overhead (the exp
                        # stream was the attention cadence limit). PV/rowsum of
                        # pair k-2 are emitted after the scores of pair k, so
                        # bank reuse has ~2 groups of slack over the exp
                        # service time. Diagonal tile v: columns [0,128v) fully
                        # masked -> skipped in scores/PV/rowsum (exp output
                        # there is garbage, never read); the triangular corner
                        # gets a mask add.
                        def emit_pv_rs(et2, ta, tb):
                            for j, t in ((0, ta), (1, tb)):
                                q0 = q0_of(t)
                                nc.tensor.matmul(ps_o[:, q0:512],
                                                 v_sb[:, b * TPB + t, :],
                                                 et2[:, j * 512 + q0:(j + 1) * 512],
                                                 start=(t == 0), stop=(t == nt - 1),
                                                 skip_group_check=True)
                                nc.tensor.matmul(ps_l[0:1, q0:512], ones_b[:, 0:1],
                                                 et2[:, j * 512 + q0:(j + 1) * 512],
                                                 start=(t == 0), stop=(t == nt - 1),
                                                 skip_group_check=True)

                        npair = nt // 2
                        pend = []   # [(et2, ta, tb)] not yet consumed by PV/rs
                        for k in range(npair):
                            ta, tb = 2 * k, 2 * k + 1
                            pst = ps_sp.tile([128, 2, 512], F32, tag="ps_s",
                                             name=f"ps_s{g}_{h}_{k}")
                            for j, t in ((0, ta), (1, tb)):
                                q0 = q0_of(t)
                                nc.tensor.matmul(pst[:, j, q0:512],
                                                 kt_sb[:, b * TPB + t, :],
                                                 qt_sb[:, g, h, q0:512],
                                                 start=True, stop=True)
                                v = t - 4 * qb
                                if v >= 0:
                                    nc.vector.tensor_add(
                                        pst[:, j, q0:q0 + 128],
                                        pst[:, j, q0:q0 + 128],
                                        masks_sb[:, v, q0:q0 + 128])
                            et2 = p2e.tile([128, 1024], BF16, tag="et2",
                                           name=f"et2{g}_{h}_{k}")
                            nc.scalar.activation(
                                et2, pst.rearrange("p a t -> p (a t)"),
                                AF.Exp, scale=SOFTMAX_SCALE)
                            pend.append((et2, ta, tb))
                            if len(pend) > 2:
                                emit_pv_rs(*pend.pop(0))
                            yield
                        for e in pend:
                            emit_pv_rs(*e)
                            yield
                        # drain ps_o fast via ACT so its single bank frees for
                        # the next head; normalize from SBUF afterwards
                        ao = p2l.tile([128, 512], BF16, tag="ao", name=f"ao{g}_{h}")
                        nc.scalar.copy(ao, ps_o)
                        lr = p2l.tile([1, 512], F32R, tag="lr", name=f"lr{g}_{h}")
                        nc.scalar.copy(lr, ps_l)
                        ps_b = ps_wp.tile([128, 512], F32, tag="ps_w",
                                          name=f"ps_b{g}_{h}")
                        nc.tensor.matmul(ps_b, ones_r[0:1, :], lr, start=True, stop=True)
                        rb = p2l.tile([128, 512], F32, tag="rb", name=f"rb{g}_{h}")
                        nc.vector.reciprocal_approx_fast(out=rb, in_=ps_b)
                        nc.vector.tensor_mul(
                            attn_t[:, h].rearrange("p r t -> p (r t)"), ao, rb)
                        yield

                def wo_block(holder):
                    b, qb, attn_t = holder["b"], holder["qb"], holder["attn_t"]
                    for r in range(4):
                        tt = b * TPB + qb * 4 + r
                        o_sb = p2.tile([128, DIM], BF16, tag="o_sb",
                                       name=f"o_sb{b}_{qb}_{r}")
                        for n in range(DIM // 512):
                            ps_w = ps_wp.tile([128, 512], F32, tag="ps_w",
                                              name=f"ps_w{b}_{qb}_{r}_{n}")
                            for kk in range(NH_LOC):
                                nc.tensor.matmul(ps_w, attn_t[:, kk, r, :],
                                                 wo_sb[:, kk, n * 512:(n + 1) * 512],
                                                 start=(kk == 0), stop=(kk == NH_LOC - 1))
                            # alternate PSUM->SBUF drain between DVE and ACT
                            if n % 2 == 0:
                                nc.vector.tensor_copy(o_sb[:, n * 512:(n + 1) * 512], ps_w)
                            else:
                                nc.scalar.copy(o_sb[:, n * 512:(n + 1) * 512], ps_w)
                            if n == 3:   # first half out early: shorter drain tail
                                nc.sync.dma_start(
                                    out=out_d[tt * 128:(tt + 1) * 128, 0:2048],
                                    in_=o_sb[:, 0:2048])
                            yield
                        nc.sync.dma_start(out=out_d[tt * 128:(tt + 1) * 128, 2048:4096],
                                          in_=o_sb[:, 2048:4096])
                        yield

                # qb descending: phase 2 opens with 36 mask-free matmuls
                # (qb=3 head 0) so the masks DMA latency is hidden
                prev_wo = None
                for b in range(B):
                    for qb in reversed(range(NQB)):
                        holder = {}
                        ag = attn_block(b, qb, holder)
                        for _ in ag:
                            if prev_wo is not None:
                                next(prev_wo, None)
                        if prev_wo is not None:
                            for _ in prev_wo:
                                pass
                        prev_wo = wo_block(holder)
                for _ in prev_wo:
                    pass

    nc.compile()
    return nc


def host_prepare(x, wq, wk, wv, wo, freqs_cos, freqs_sin, B, S):
    """Build per-core in_maps. Weights nn.Linear-style [out, in]."""
    NQB = S // 512
    NTG = B * NQB
    n_heads = wq.shape[0] // HD
    n_kv = wk.shape[0] // HD
    hpc = n_heads // N_CORES       # q heads per core (4)
    kpc = n_kv // N_CORES          # kv heads per core (1)

    # deinterleave rope pairs: feature order (2i) first then (2i+1), per head
    de = np.concatenate([np.arange(0, HD, 2), np.arange(1, HD, 2)])

    xf = np.ascontiguousarray(x.reshape(B * S, DIM))
    # x^T tiled, partition-contiguous: [g, kq, p, k, t]
    x_t = np.ascontiguousarray(
        xf.T.reshape(NKQ, KPQ, 128, NTG, 512).transpose(3, 0, 2, 1, 4)).astype(NPBF16)

    cos = np.repeat(freqs_cos, 2, axis=1)   # [S, 128] interleaved dup
    sin = np.repeat(freqs_sin, 2, axis=1)
    cc = cos[:, de]                                             # deinterleaved
    ss = sin.copy()
    ss[:, 1::2] *= -1.0                     # half-swapped: [+sin; -sin]
    ss = ss[:, de]
    cct = np.ascontiguousarray(cc.T).astype(NPBF16)             # [128, S]
    sst = np.ascontiguousarray(ss.T).astype(NPBF16)

    ident = np.eye(128, dtype=NPBF16)
    ones = np.ones((128, 128), dtype=np.float32)
    onesb = np.ones((128, 128), dtype=NPBF16)
    # transposed-orientation causal masks: scores^T [sk within tile, sq in 512]
    r_idx = np.arange(128)[:, None]
    j_idx = np.arange(512)[None, :]
    masks = np.ascontiguousarray(np.stack([
        np.where(v * 128 + r_idx <= j_idx, 0.0, -1e30).astype(np.float32)
        for v in range(4)]).transpose(1, 0, 2))   # [128, 4, 512]

    in_maps = []
    for cidx in range(N_CORES):
        qs = slice(cidx * hpc * HD, (cidx + 1) * hpc * HD)
        ks = slice(cidx * kpc * HD, (cidx + 1) * kpc * HD)
        wq_c = wq[qs].reshape(hpc, HD, DIM)[:, de, :].reshape(hpc * HD, DIM)
        wk_c = wk[ks].reshape(kpc, HD, DIM)[:, de, :].reshape(kpc * HD, DIM)
        wv_c = wv[ks]
        wkv_c = np.concatenate([wk_c, wv_c], axis=0)
        wo_c = wo[:, qs]
        # weights [out, in] -> [p, ktile, out] partition-contiguous
        wq_pk = np.ascontiguousarray(
            wq_c.T.reshape(NK, 128, hpc * HD).transpose(1, 0, 2)).astype(NPBF16)
        wkv_pk = np.ascontiguousarray(
            wkv_c.T.reshape(NK, 128, KVDIM).transpose(1, 0, 2)).astype(NPBF16)
        in_maps.append({
            "x_t": x_t,
            "wq_t": wq_pk,
            "wkv_t": wkv_pk,
            "wo_t": np.ascontiguousarray(wo_c.T).astype(NPBF16),
            "cct": cct,
            "sst": sst,
            "ident": ident,
            "ones": ones,
            "onesb": onesb,
            "masks": masks,
        })
    return in_maps


_CACHE = {}


def run(inputs, trace=False, trace_cores=None):
    x = np.asarray(inputs["x"], dtype=np.float32)
    B, S, _ = x.shape
    key = (B, S)
    if key not in _CACHE:
        _CACHE[key] = build_nc(B, S)
    nc = _CACHE[key]
    in_maps = host_prepare(
        x, np.asarray(inputs["wq"], np.float32), np.asarray(inputs["wk"], np.float32),
        np.asarray(inputs["wv"], np.float32), np.asarray(inputs["wo"], np.float32),
        np.asarray(inputs["freqs_cos"], np.float32),
        np.asarray(inputs["freqs_sin"], np.float32), B, S)
    res = bass_utils.run_bass_kernel_spmd(
        nc, in_maps, core_ids=list(range(N_CORES)), trace=trace,
        trace_cores=trace_cores)
    acc = np.zeros((B * S, DIM), dtype=np.float64)
    for r in res.results:
        acc += r["out"].astype(np.float64)
    out = acc.astype(np.float32).reshape(B, S, DIM)
    return out, res


def kernel(**inputs) -> np.ndarray:
    assert int(inputs.get("start_pos", 0)) == 0
    out, _ = run(inputs, trace=False)
    return out


# revision 53
# speedup vs baseline: 1.0229x; 1.0025x over previous
"""Trainium2 Bass kernel for nn_Attention_38276748542551.

Llama-style GQA attention block (DIM=4096, 32 q-heads, 8 kv-heads, hd=128,
b=2, s=2048, start_pos=0), tensor-parallel over heads across 8 NeuronCores:
each core owns 4 q-heads / 1 kv-head (wq/wk/wv output-dim shard, wo
input-dim shard) and computes a full [b*s, 4096] partial of the wo output;
the all-reduce is done on the host after gathering the 8 partials (partials
are written bf16 to halve the output DMA).

All matmuls run in bf16 (same 1 col/cycle PE rate as float32r but half the
LDWEIGHTS/SBUF/DMA traffic); accumulation is fp32 in PSUM. Measured rel-err
budget: bf16 operand quantization ~2e-3 vs the 2e-2 gate.

Device dataflow per core (everything feature-major, moving dim = 512 tokens):
  phase 1 (per 512-token block): Q/K/V projections with the weight k-tile
  stationary and x^T (host pre-transposed, bf16) moving -> outputs land
  [feat, tok] in PSUM -> rope applied by the DVE reading PSUM directly: the
  pair-swap is a partition half-rotation (even/odd deinterleave baked into
  the weight layout), done with partition-offset [64,512] multiplies -- no
  PE swap matmul, no ACT copy. Q^T goes to a resident SBUF tile (no DRAM
  roundtrip), K^T resident in SBUF, V PE-transposed to token-major bf16.
  phase 2 (per batch, per 512-token query block, per head): scores computed
  TRANSPOSED [sk, sq] per 128-sk-tile (lhsT = K^T tile, rhs = Q^T block,
  N=512) -> causal mask add on diagonal-band tiles -> exp (ACT, bf16 out) ->
  PV accumulation (lhsT = V tile, N=512) and row-sum accumulation (lhsT =
  ones column) on the PE -> denominator broadcast via a K=1 ones-matmul ->
  approx-reciprocal + multiply -> attn^T bf16, feature-major.
  phase 3 (per 128-token tile): wo projection (lhsT = attn^T, rhs = wo^T,
  8x N=512 chunks x 4 k-tiles) -> PSUM->SBUF copies alternate between the
  scalar and vector engines -> bf16 partial DMA'd out.
"""
import sys
import numpy as np
import ml_dtypes

sys.path.insert(0, "/opt/trn_rl_repo")

import concourse.bass as bass  # noqa: E402
import concourse.tile as tile  # noqa: E402
from concourse import bacc, mybir  # noqa: E402
from concourse import bass_utils  # noqa: E402

F32 = mybir.dt.float32
F32R = mybir.dt.float32r
BF16 = mybir.dt.bfloat16
AF = mybir.ActivationFunctionType
NPBF16 = ml_dtypes.bfloat16

DIM = 4096
NK = DIM // 128          # contraction k-tiles (32)
NKQ = 4                  # k quarters
KPQ = NK // NKQ          # k-tiles per quarter (8)
HD = 128                 # head dim
NH_LOC = 4               # q heads per core
QDIM = NH_LOC * HD       # 512
KVDIM = 2 * HD           # K and V projected together, 256
N_CORES = 8
SOFTMAX_SCALE = 1.0 / np.sqrt(HD)


def build_nc(B=2, S=2048):
    """Build the per-core Bass program (identical across cores; data differs)."""
    NT = B * S // 128            # 128-token tiles total
    TPB = S // 128               # 128-token tiles per batch
    NQB = S // 512               # 512-token blocks per batch
    NTG = B * NQB                # 512-token blocks total

    nc = bacc.Bacc("TRN2", target_bir_lowering=False, debug=False,
                   enable_asserts=False, num_devices=N_CORES)

    # x_t laid out partition-contiguous on host: [g, kq, p, k, t]
    x_t = nc.dram_tensor("x_t", [NTG, NKQ, 128, KPQ, 512], BF16, kind="ExternalInput").ap()
    # weights laid out [p, k, n] on host so DMA lines are contiguous per partition
    wq_t = nc.dram_tensor("wq_t", [128, NK, QDIM], BF16, kind="ExternalInput").ap()
    wkv_t = nc.dram_tensor("wkv_t", [128, NK, KVDIM], BF16, kind="ExternalInput").ap()
    wo_t = nc.dram_tensor("wo_t", [QDIM, DIM], BF16, kind="ExternalInput").ap()
    cct_d = nc.dram_tensor("cct", [128, S], BF16, kind="ExternalInput").ap()
    sst_d = nc.dram_tensor("sst", [128, S], BF16, kind="ExternalInput").ap()
    ident_d = nc.dram_tensor("ident", [128, 128], BF16, kind="ExternalInput").ap()
    ones_d = nc.dram_tensor("ones", [128, 128], F32, kind="ExternalInput").ap()
    onesb_d = nc.dram_tensor("onesb", [128, 128], BF16, kind="ExternalInput").ap()
    masks_d = nc.dram_tensor("masks", [128, 4, 512], F32, kind="ExternalInput").ap()
    out_d = nc.dram_tensor("out", [B * S, DIM], BF16, kind="ExternalOutput").ap()

    with tile.TileContext(nc) as tc:
        with tc.tile_pool(name="singles", bufs=1) as singles:
            ident = singles.tile([128, 128], BF16)
            ones_r = singles.tile([128, 128], F32R)
            ones_b = singles.tile([128, 128], BF16)
            kt_sb = singles.tile([128, NT, 128], BF16)    # K^T: [hd, tile, tok]
            v_sb = singles.tile([128, NT, 128], BF16)     # V: [tok, tile, hd]
            qt_sb = singles.tile([128, NTG, NH_LOC, 512], BF16)  # Q^T resident
            cct_sb = singles.tile([128, S], BF16)
            sst_sb = singles.tile([128, S], BF16)

            # ---------------- phase 1: projections + rope (feature-major) ----------------
            with tc.tile_pool(name="p1w", bufs=1) as p1w, \
                 tc.tile_pool(name="p1", bufs=4) as p1, \
                 tc.tile_pool(name="p1r", bufs=3) as p1r, \
                 tc.tile_pool(name="ps_acc", bufs=6, space="PSUM") as ps_accp, \
                 tc.tile_pool(name="ps_misc", bufs=2, space="PSUM") as ps_miscp:

                def load_xs(g, kq):
                    t_ = p1.tile([128, KPQ, 512], BF16, tag="xs")
                    nc.sync.dma_start(out=t_, in_=x_t[g, kq])
                    return t_

                wq_sb = p1w.tile([128, NK, QDIM], BF16)
                wkv_sb = p1w.tile([128, NK, KVDIM], BF16)

                def load_wq(k0, k1):
                    nc.sync.dma_start(
                        out=wq_sb[:, k0:k1, :], in_=wq_t[:, k0:k1, :])

                def load_wkv(k0, k1):
                    nc.sync.dma_start(
                        out=wkv_sb[:, k0:k1, :], in_=wkv_t[:, k0:k1, :])

                # interleave the first xs quarter with the weight chunk loads so
                # the first matmul can start as early as possible
                xs00 = p1.tile([128, KPQ, 512], BF16, tag="xs")
                nc.sync.dma_start(out=xs00[:, 0:1, :], in_=x_t[0, 0, :, 0:1])
                load_wq(0, 1)
                load_wkv(0, 1)
                nc.sync.dma_start(out=xs00[:, 1:3, :], in_=x_t[0, 0, :, 1:3])
                load_wq(1, 3)
                load_wkv(1, 3)
                nc.sync.dma_start(out=xs00[:, 3:5, :], in_=x_t[0, 0, :, 3:5])
                load_wq(3, 5)
                load_wkv(3, 5)
                nc.sync.dma_start(out=xs00[:, 5:8, :], in_=x_t[0, 0, :, 5:8])
                load_wq(5, 8)
                load_wkv(5, 8)
                # deadline-ordered: each xs quarter + the weight k-tiles it
                # needs, so concurrent DMA queues don't starve the urgent ones
                xs_pre = [xs00]
                xs_pre.append(load_xs(0, 1))
                load_wq(8, 16)
                load_wkv(8, 16)
                xs_pre.append(load_xs(0, 2))
                load_wq(16, 24)
                load_wkv(16, 24)
                load_wq(24, 32)
                load_wkv(24, 32)
                nc.sync.dma_start(out=cct_sb, in_=cct_d)
                nc.sync.dma_start(out=sst_sb, in_=sst_d)
                nc.sync.dma_start(out=ident, in_=ident_d)
                nc.sync.dma_start(out=ones_r, in_=ones_d.bitcast(F32R))
                nc.sync.dma_start(out=ones_b, in_=onesb_d)

                nload = 3   # next (g*NKQ+kq) index to load; keep 2-3 in flight
                for g in range(NTG):
                    pos = (g % NQB) * 512
                    acc = [ps_accp.tile([128, 512], F32, tag="acc", name=f"acc{g}_{j}") for j in range(6)]
                    for kq in range(NKQ):
                        xs = xs_pre.pop(0)
                        if nload < NTG * NKQ:
                            xs_pre.append(load_xs(nload // NKQ, nload % NKQ))
                            nload += 1
                        for k in range(KPQ):
                            kt = kq * KPQ + k
                            st = (kt == 0)
                            sp = (kt == NK - 1)
                            for h in range(NH_LOC):
                                nc.tensor.matmul(acc[h], wq_sb[:, kt, h * 128:(h + 1) * 128],
                                                 xs[:, k, :], start=st, stop=sp)
                            nc.tensor.matmul(acc[4], wkv_sb[:, kt, 0:128],
                                             xs[:, k, :], start=st, stop=sp)
                            nc.tensor.matmul(acc[5], wkv_sb[:, kt, 128:256],
                                             xs[:, k, :], start=st, stop=sp)

                    # Free the PSUM accumulators FAST with split ACT/DVE bf16
                    # staging copies (so the next block's matmuls aren't blocked
                    # on the rope chain), then rope from the bf16 staging.
                    stage = [p1r.tile([128, 512], BF16, tag=f"st{j}", name=f"st{g}_{j}")
                             for j in range(6)]
                    for j in range(6):
                        if j % 2 == 0:
                            nc.scalar.copy(stage[j], acc[j])
                        else:
                            nc.vector.tensor_copy(stage[j], acc[j])
                    # rope Q (4 heads) + K on the DVE in bf16 (2x mode).
                    # feature layout per head: [0:64]=even pairs, [64:128]=odd.
                    # out_e = e*c - o*s ; out_o = o*c + e*s. sst is stored
                    # half-swapped ([+s; -s]) so each multiply reads both SBUF
                    # inputs at the same base partition (HW constraint) and the
                    # half-rotation happens via the shifted output write.
                    cct = cct_sb[:, pos:pos + 512]
                    sst = sst_sb[:, pos:pos + 512]
                    for j in range(5):   # 0..3 = q heads, 4 = K
                        t1 = p1r.tile([128, 512], BF16, tag="t1")
                        nc.vector.tensor_mul(t1, stage[j], cct)
                        t2 = p1r.tile([128, 512], BF16, tag="t2")
                        nc.vector.tensor_mul(t2[0:64, :], stage[j][64:128, :], sst[64:128, :])
                        nc.vector.tensor_mul(t2[64:128, :], stage[j][0:64, :], sst[0:64, :])
                        if j < NH_LOC:
                            nc.vector.tensor_add(qt_sb[:, g, j, :], t1, t2)
                        else:
                            nc.vector.tensor_add(
                                kt_sb[:, 4 * g:4 * g + 4, :].rearrange("p a t -> p (a t)"),
                                t1, t2)
                    for r in range(4):
                        ps_vt = ps_miscp.tile([128, 512], BF16, tag="misc")
                        nc.tensor.transpose(ps_vt[:, 0:128], stage[5][:, r * 128:(r + 1) * 128], ident)
                        nc.scalar.copy(v_sb[:, 4 * g + r, :], ps_vt[:, 0:128])

            # ------------- phase 2/3: attention (transposed scores) + wo -------------
            with tc.tile_pool(name="p2w", bufs=1) as p2w, \
                 tc.tile_pool(name="p2", bufs=2) as p2, \
                 tc.tile_pool(name="p2e", bufs=4) as p2e, \
                 tc.tile_pool(name="p2l", bufs=4) as p2l, \
                 tc.tile_pool(name="ps_s", bufs=2, space="PSUM") as ps_sp, \
                 tc.tile_pool(name="ps_o", bufs=1, space="PSUM") as ps_op, \
                 tc.tile_pool(name="ps_l", bufs=1, space="PSUM") as ps_lp, \
                 tc.tile_pool(name="ps_w", bufs=2, space="PSUM") as ps_wp:
                masks_sb = p2w.tile([128, 4, 512], F32)
                nc.sync.dma_start(out=masks_sb, in_=masks_d)
                # one PSUM bank, two rowsum rows: heads alternate rows so a
                # head's first rowsum never waits on the previous head's lr copy
                ps_l2 = ps_lp.tile([64, 512], F32, tag="ps_l")
                wo_sb = p2w.tile([128, NH_LOC, DIM], BF16)
                for kk in range(NH_LOC):   # chunked so the first wo can start early
                    nc.sync.dma_start(
                        out=wo_sb[:, kk, :],
                        in_=wo_t[kk * 128:(kk + 1) * 128, :])

                # Attention and wo are emitted as generators and interleaved:
                # each block's attention steps alternate with the PREVIOUS
                # block's wo chunks so the scheduler's priorities (emission
                # order) match the intended concurrency.
                def attn_block(b, qb, holder):
                    g = b * NQB + qb
                    nt = 4 * (qb + 1)
                    attn_t = p2.tile([128, NH_LOC, 4, 128], BF16, tag="attn_t",
                                     name=f"attn_t{g}")
                    holder["attn_t"] = attn_t
                    holder["b"] = b
                    holder["qb"] = qb
                    def q0_of(t):
                        v = t - 4 * qb
                        return 128 * v if v > 0 else 0

                    for h in range(NH_LOC):
                        ps_o = ps_op.tile([128, 512], F32, tag="ps_o",
                                          name=f"ps_o{g}_{h}")
                        lrow = 32 * (h % 2)
                        ps_l = ps_l2[lrow:lrow + 1, :]

                        # k-tiles in PAIRS: one exp per pair over a 2-bank
                        # [128,1024] PSUM read halves the ACT overhead (the exp
                        # stream was the attention cadence limit). PV/rowsum of
                        # pair k-2 are emitted after the scores of pair k, so
                        # bank reuse has ~2 groups of slack over the exp
                        # service time. Diagonal tile v: columns [0,128v) fully
                        # masked -> skipped in scores/PV/rowsum (exp output
                        # there is garbage, never read); the triangular corner
                        # gets a mask add.
                        def emit_pv_rs(et2, ta, tb):
                            for j, t in ((0, ta), (1, tb)):
                                q0 = q0_of(t)
                                nc.tensor.matmul(ps_o[:, q0:512],
                                                 v_sb[:, b * TPB + t, :],
                                                 et2[:, j * 512 + q0:(j + 1) * 512],
                                                 start=(t == 0), stop=(t == nt - 1),
                                                 skip_group_check=True)
                                nc.tensor.matmul(ps_l[0:1, q0:512], ones_b[:, 0:1],
                                                 et2[:, j * 512 + q0:(j + 1) * 512],
                                                 start=(t == 0), stop=(t == nt - 1),
                                                 skip_group_check=True)

                        npair = nt // 2
                        pend = []   # [(et2, ta, tb)] not yet consumed by PV/rs
                        for k in range(npair):
                            ta, tb = 2 * k, 2 * k + 1
                            pst = ps_sp.tile([128, 2, 512], F32, tag="ps_s",
                                             name=f"ps_s{g}_{h}_{k}")
                            for j, t in ((0, ta), (1, tb)):
                                q0 = q0_of(t)
                                nc.tensor.matmul(pst[:, j, q0:512],
                                                 kt_sb[:, b * TPB + t, :],
                                                 qt_sb[:, g, h, q0:512],
                                                 start=True, stop=True)
                                v = t - 4 * qb
                                if v >= 0:
                                    nc.vector.tensor_add(
                                        pst[:, j, q0:q0 + 128],
                                        pst[:, j, q0:q0 + 128],
                                        masks_sb[:, v, q0:q0 + 128])
                            et2 = p2e.tile([128, 1024], BF16, tag="et2",
                                           name=f"et2{g}_{h}_{k}")
                            nc.scalar.activation(
                                et2, pst.rearrange("p a t -> p (a t)"),
                                AF.Exp, scale=SOFTMAX_SCALE)
                            pend.append((et2, ta, tb))
                            if len(pend) > 2:
                                emit_pv_rs(*pend.pop(0))
                            yield
                        for e in pend:
                            emit_pv_rs(*e)
                            yield
                        # drain ps_o fast via ACT so its single bank frees for
                        # the next head; normalize from SBUF afterwards
                        ao = p2l.tile([128, 512], BF16, tag="ao", name=f"ao{g}_{h}")
                        nc.scalar.copy(ao, ps_o)
                        lr = p2l.tile([1, 512], F32R, tag="lr", name=f"lr{g}_{h}")
                        nc.scalar.copy(lr, ps_l)
                        ps_b = ps_wp.tile([128, 512], F32, tag="ps_w",
                                          name=f"ps_b{g}_{h}")
                        nc.tensor.matmul(ps_b, ones_r[0:1, :], lr, start=True, stop=True)
                        rb = p2l.tile([128, 512], F32, tag="rb", name=f"rb{g}_{h}")
                        nc.vector.reciprocal_approx_fast(out=rb, in_=ps_b)
                        nc.vector.tensor_mul(
                            attn_t[:, h].rearrange("p r t -> p (r t)"), ao, rb)
                        yield

                def wo_block(holder):
                    b, qb, attn_t = holder["b"], holder["qb"], holder["attn_t"]
                    for r in range(4):
                        tt = b * TPB + qb * 4 + r
                        o_sb = p2.tile([128, DIM], BF16, tag="o_sb",
                                       name=f"o_sb{b}_{qb}_{r}")
                        for n in range(DIM // 512):
                            ps_w = ps_wp.tile([128, 512], F32, tag="ps_w",
                                              name=f"ps_w{b}_{qb}_{r}_{n}")
                            for kk in range(NH_LOC):
                                nc.tensor.matmul(ps_w, attn_t[:, kk, r, :],
                                                 wo_sb[:, kk, n * 512:(n + 1) * 512],
                                                 start=(kk == 0), stop=(kk == NH_LOC - 1))
                            # alternate PSUM->SBUF drain between DVE and ACT
                            if n % 2 == 0:
                                nc.vector.tensor_copy(o_sb[:, n * 512:(n + 1) * 512], ps_w)
                            else:
                                nc.scalar.copy(o_sb[:, n * 512:(n + 1) * 512], ps_w)
                            if n % 2 == 1 and n < 7:   # stream quarters out early
                                nc.sync.dma_start(
                                    out=out_d[tt * 128:(tt + 1) * 128,
                                              (n - 1) * 512:(n + 1) * 512],
                                    in_=o_sb[:, (n - 1) * 512:(n + 1) * 512])
                            yield
                        nc.sync.dma_start(out=out_d[tt * 128:(tt + 1) * 128, 3072:4096],
                                          in_=o_sb[:, 3072:4096])
                        yield

                # qb descending: phase 2 opens with 36 mask-free matmuls
                # (qb=3 head 0) so the masks DMA latency is hidden
                prev_wo = None
                for b in range(B):
                    for qb in reversed(range(NQB)):
                        holder = {}
                        ag = attn_block(b, qb, holder)
                        for _ in ag:
                            if prev_wo is not None:
                                next(prev_wo, None)
                        if prev_wo is not None:
                            for _ in prev_wo:
                                pass
                        prev_wo = wo_block(holder)
                for _ in prev_wo:
                    pass

    nc.compile()
    return nc


def host_prepare(x, wq, wk, wv, wo, freqs_cos, freqs_sin, B, S):
    """Build per-core in_maps. Weights nn.Linear-style [out, in]."""
    NQB = S // 512
    NTG = B * NQB
    n_heads = wq.shape[0] // HD
    n_kv = wk.shape[0] // HD
    hpc = n_heads // N_CORES       # q heads per core (4)
    kpc = n_kv // N_CORES          # kv heads per core (1)

    # deinterleave rope pairs: feature order (2i) first then (2i+1), per head
    de = np.concatenate([np.arange(0, HD, 2), np.arange(1, HD, 2)])

    xf = np.ascontiguousarray(x.reshape(B * S, DIM))
    # x^T tiled, partition-contiguous: [g, kq, p, k, t]
    x_t = np.ascontiguousarray(
        xf.T.reshape(NKQ, KPQ, 128, NTG, 512).transpose(3, 0, 2, 1, 4)).astype(NPBF16)

    cos = np.repeat(freqs_cos, 2, axis=1)   # [S, 128] interleaved dup
    sin = np.repeat(freqs_sin, 2, axis=1)
    cc = cos[:, de]                                             # deinterleaved
    ss = sin.copy()
    ss[:, 1::2] *= -1.0                     # half-swapped: [+sin; -sin]
    ss = ss[:, de]
    cct = np.ascontiguousarray(cc.T).astype(NPBF16)             # [128, S]
    sst = np.ascontiguousarray(ss.T).astype(NPBF16)

    ident = np.eye(128, dtype=NPBF16)
    ones = np.ones((128, 128), dtype=np.float32)
    onesb = np.ones((128, 128), dtype=NPBF16)
    # transposed-orientation causal masks: scores^T [sk within tile, sq in 512]
    r_idx = np.arange(128)[:, None]
    j_idx = np.arange(512)[None, :]
    masks = np.ascontiguousarray(np.stack([
        np.where(v * 128 + r_idx <= j_idx, 0.0, -1e30).astype(np.float32)
        for v in range(4)]).transpose(1, 0, 2))   # [128, 4, 512]

    in_maps = []
    for cidx in range(N_CORES):
        qs = slice(cidx * hpc * HD, (cidx + 1) * hpc * HD)
        ks = slice(cidx * kpc * HD, (cidx + 1) * kpc * HD)
        wq_c = wq[qs].reshape(hpc, HD, DIM)[:, de, :].reshape(hpc * HD, DIM)
        wk_c = wk[ks].reshape(kpc, HD, DIM)[:, de, :].reshape(kpc * HD, DIM)
        wv_c = wv[ks]
        wkv_c = np.concatenate([wk_c, wv_c], axis=0)
        wo_c = wo[:, qs]
        # weights [out, in] -> [p, ktile, out] partition-contiguous
        wq_pk = np.ascontiguousarray(
            wq_c.T.reshape(NK, 128, hpc * HD).transpose(1, 0, 2)).astype(NPBF16)
        wkv_pk = np.ascontiguousarray(
            wkv_c.T.reshape(NK, 128, KVDIM).transpose(1, 0, 2)).astype(NPBF16)
        in_maps.append({
            "x_t": x_t,
            "wq_t": wq_pk,
            "wkv_t": wkv_pk,
            "wo_t": np.ascontiguousarray(wo_c.T).astype(NPBF16),
            "cct": cct,
            "sst": sst,
            "ident": ident,
            "ones": ones,
            "onesb": onesb,
            "masks": masks,
        })
    return in_maps


_CACHE = {}


def run(inputs, trace=False, trace_cores=None):
    x = np.asarray(inputs["x"], dtype=np.float32)
    B, S, _ = x.shape
    key = (B, S)
    if key not in _CACHE:
        _CACHE[key] = build_nc(B, S)
    nc = _CACHE[key]
    in_maps = host_prepare(
        x, np.asarray(inputs["wq"], np.float32), np.asarray(inputs["wk"], np.float32),
        np.asarray(inputs["wv"], np.float32), np.asarray(inputs["wo"], np.float32),
        np.asarray(inputs["freqs_cos"], np.float32),
        np.asarray(inputs["freqs_sin"], np.float32), B, S)
    res = bass_utils.run_bass_kernel_spmd(
        nc, in_maps, core_ids=list(range(N_CORES)), trace=trace,
        trace_cores=trace_cores)
    acc = np.zeros((B * S, DIM), dtype=np.float64)
    for r in res.results:
        acc += r["out"].astype(np.float64)
    out = acc.astype(np.float32).reshape(B, S, DIM)
    return out, res


def kernel(**inputs) -> np.ndarray:
    assert int(inputs.get("start_pos", 0)) == 0
    out, _ = run(inputs, trace=False)
    return out
